# revision 33
# baseline (speedup 1.0000x reference)
"""AGNN (3-layer) Trainium2 Bass kernel, 8-core SPMD.

Design:
  dst-node sharding (6250 real nodes/core, NSH=6272 padded positions).
  Gather table = raw node features (bf16, 256B rows) in compact global
  order, AllGathered per layer (12.85 MB). Per-edge dot AND source
  sumsq recomputed on DVE from gathered raw rows (interleaved P/Q
  shared halving tree), so no [xn|xu] 512B rows are needed.
  Self-loops handled analytically (score = beta exactly -> exp(beta)
  terms in numerator/denominator; no slots, no gather).
  Softmax without max-subtraction (|alpha| <= beta).
  Node layout: per-core snake sort by (kA, kB) degree classes over
  {1,2,3,4,6,8,12,16,24,32}; 128-node blocks take per-block max class,
  unified across cores (one NEFF). A/B source halves (4 cores each)
  keep gather indices within int16.
  y stays in SBUF between layers; repack perm->compact via dma_gather
  through DRAM at layer end feeds the next AllGather.
"""

import numpy as np
import sys, os
from contextlib import ExitStack

for _p in ("/opt/trn_rl_repo", "/root/.axon_site/_ro/trn_rl_repo"):
    if os.path.isdir(_p) and _p not in sys.path:
        try:
            import concourse  # noqa
            break
        except Exception:
            sys.path.insert(0, _p)

NCORE = 8
N = 50000
D = 128
NSH_REAL = 6250
NBLK = 49
NSH = NBLK * 128  # 6272
HALF = 4 * NSH  # 25088
NTAB = 8 * NSH  # 50176
KC = np.array([1, 2, 3, 4, 6, 8, 12, 16, 24, 32], dtype=np.int64)
CHMAX = 48  # max gather cols (128 slots each) per bucket per call
NEG = np.float32(-1e30)


def _classes(d):
    return KC[np.searchsorted(KC, np.maximum(d, 1))]


def _plan(edge_index):
    src = np.ascontiguousarray(edge_index[0]).astype(np.int64)
    dst = np.ascontiguousarray(edge_index[1]).astype(np.int64)
    core = dst // NSH_REAL
    loc = dst - core * NSH_REAL
    isA = src < HALF // NSH * NSH_REAL  # src < 25000

    key = core * NSH_REAL + loc
    dA = np.bincount(key[isA], minlength=N).reshape(NCORE, NSH_REAL)
    dB = np.bincount(key[~isA], minlength=N).reshape(NCORE, NSH_REAL)
    kA = _classes(dA)
    kB = _classes(dB)

    # snake order per core: kA asc, kB asc/desc alternating by kA class idx
    kci = np.zeros(int(KC[-1]) + 1, dtype=np.int64)
    kci[KC] = np.arange(len(KC))
    orders = np.empty((NCORE, NSH_REAL), dtype=np.int64)
    blocksA = np.zeros((NCORE, NBLK), dtype=np.int64)
    blocksB = np.zeros((NCORE, NBLK), dtype=np.int64)
    for c in range(NCORE):
        snake_kb = np.where((kci[kA[c]] % 2) == 1, -kB[c], kB[c])
        o = np.lexsort((snake_kb, kA[c]))
        orders[c] = o
        kAs = np.zeros(NSH, dtype=np.int64)
        kBs = np.zeros(NSH, dtype=np.int64)
        kAs[:NSH_REAL] = kA[c][o]
        kBs[:NSH_REAL] = kB[c][o]
        blocksA[c] = kAs.reshape(NBLK, 128).max(1)
        blocksB[c] = kBs.reshape(NBLK, 128).max(1)
    bA = blocksA.max(0)
    bB = blocksB.max(0)

    colbaseA = np.concatenate([[0], np.cumsum(bA)])
    colbaseB = np.concatenate([[0], np.cumsum(bB)])
    CA, CB = int(colbaseA[-1]), int(colbaseB[-1])
    calls = []
    b = 0
    while b < NBLK:
        e = b + 1
        while e < NBLK and bA[e] == bA[b] and bB[e] == bB[b]:
            e += 1
        kmax = max(bA[b], bB[b])
        npc = max(int(CHMAX // kmax), 1)
        for s in range(b, e, npc):
            nb = min(npc, e - s)
            calls.append(dict(
                ka=int(bA[b]), kb=int(bB[b]), b0=int(s), nb=int(nb),
                colA0=int(colbaseA[s]), colB0=int(colbaseB[s]),
                colsA=int(bA[b] * nb), colsB=int(bB[b] * nb),
            ))
        b = e

    SA, SB = CA * 128, CB * 128
    per_core = []
    for c in range(NCORE):
        o = orders[c]
        pos = np.empty(NSH_REAL, dtype=np.int64)
        pos[o] = np.arange(NSH_REAL)
        m = core == c
        e_src = src[m]
        e_loc = loc[m]
        e_isA = isA[m]
        q = e_loc * 2 + (~e_isA)
        s_idx = np.argsort(q, kind="stable")
        qs = q[s_idx]
        newgrp = np.ones(len(qs), dtype=bool)
        if len(qs) > 1:
            newgrp[1:] = qs[1:] != qs[:-1]
        starts = np.nonzero(newgrp)[0]
        grp_id = np.cumsum(newgrp) - 1
        rank_sorted = np.arange(len(qs)) - starts[grp_id]
        rank = np.empty(len(qs), dtype=np.int64)
        rank[s_idx] = rank_sorted
        p = pos[e_loc]
        blk = p // 128
        part = p - blk * 128
        src_core = e_src // NSH_REAL
        src_u = e_src - src_core * NSH_REAL
        tabrow = src_core * NSH + src_u
        idxA = np.zeros(SA, dtype=np.int64)
        idxB = np.zeros(SB, dtype=np.int64)
        maskA = np.full(SA, NEG, dtype=np.float32)
        maskB = np.full(SB, NEG, dtype=np.float32)
        a = e_isA
        iA = (colbaseA[blk[a]] + rank[a]) * 128 + part[a]
        idxA[iA] = tabrow[a]
        maskA[iA] = 0.0
        nb_ = ~a
        iB = (colbaseB[blk[nb_]] + rank[nb_]) * 128 + part[nb_]
        idxB[iB] = tabrow[nb_] - HALF
        maskB[iB] = 0.0
        idx0 = np.zeros(NSH, dtype=np.int64)
        idx0[:NSH_REAL] = o
        idxR = np.zeros(NSH, dtype=np.int64)
        idxR[:NSH_REAL] = pos

        def wrap16(v):
            return v.reshape(-1, 16).T.astype(np.int16)

        idxM = np.concatenate(
            [wrap16(idxA), wrap16(idxB), wrap16(idx0), wrap16(idxR)], axis=1)
        maskM = np.concatenate(
            [maskA.reshape(-1, 128).T, maskB.reshape(-1, 128).T],
            axis=1).astype(np.float32)
        per_core.append(dict(idxM=np.ascontiguousarray(idxM),
                             maskM=np.ascontiguousarray(maskM)))
    meta = dict(CA=CA, CB=CB, W=per_core[0]["idxM"].shape[1], calls=calls)
    return per_core, meta


def _build_nc(meta):
    import concourse.bass as bass
    import concourse.bacc as bacc
    import concourse.tile as tile
    from concourse import mybir, library_config

    f32, bf, i16 = mybir.dt.float32, mybir.dt.bfloat16, mybir.dt.int16
    Alu = mybir.AluOpType
    Act = mybir.ActivationFunctionType

    CA, CB, W = meta["CA"], meta["CB"], meta["W"]
    CM = CA + CB
    OFF0 = CM * 8
    OFFR = OFF0 + NSH // 16

    nc = bacc.Bacc("TRN2", target_bir_lowering=False, debug=False,
                   num_devices=NCORE, num_swdge_queues=1,
                   dynamic_dma_scratch_size=24576)
    x_tab_d = nc.dram_tensor("x_tab", [NSH, D], bf, kind="ExternalInput")
    idxM_d = nc.dram_tensor("idxM", [16, W], i16, kind="ExternalInput")
    maskM_d = nc.dram_tensor("maskM", [128, CM], f32, kind="ExternalInput")
    beta_d = nc.dram_tensor("betas", [1, 4], f32, kind="ExternalInput")
    y_d = nc.dram_tensor("y", [NSH, D], bf, kind="ExternalOutput")

    with ExitStack() as ctx:
        tc = ctx.enter_context(tile.TileContext(nc))
        res = ctx.enter_context(tc.tile_pool(name="res", bufs=1))
        dram = ctx.enter_context(tc.tile_pool(name="dram", bufs=1, space="DRAM"))
        gat = ctx.enter_context(tc.tile_pool(name="gat", bufs=2))
        tq = ctx.enter_context(tc.tile_pool(name="tq", bufs=2))
        sm = ctx.enter_context(tc.tile_pool(name="sm", bufs=4))
        vp = ctx.enter_context(tc.tile_pool(name="vp", bufs=2))
        nrm = ctx.enter_context(tc.tile_pool(name="nrm", bufs=1))
        ys = ctx.enter_context(tc.tile_pool(name="ys", bufs=2))

        idx_s = res.tile([128, W], i16)
        mask_s = res.tile([128, CM], f32)
        beta_s = res.tile([128, 4], f32)
        expb_s = res.tile([128, 4], f32)
        eps_s = res.tile([128, 1], f32)
        tab_shard = dram.tile([NSH, D], bf)
        tab_fulls = [dram.tile([NTAB, D], bf, addr_space="Shared",
                               name=f"tab_full{i}") for i in range(3)]
        y_dram = dram.tile([NSH, D], bf)

        nc.gpsimd.load_library(library_config.mlp)
        for b in range(8):
            nc.sync.dma_start(out=idx_s[16 * b : 16 * (b + 1), :], in_=idxM_d[:])
        nc.sync.dma_start(out=mask_s[:], in_=maskM_d[:])
        bap = beta_d[:]
        nc.gpsimd.dma_start(
            out=beta_s[:],
            in_=bass.AP(tensor=bap.tensor, offset=bap.offset, ap=[[0, 128], [1, 4]]),
        )
        nc.vector.memset(eps_s[:], 1e-20)
        nc.scalar.activation(out=expb_s[:], in_=beta_s[:], func=Act.Exp)

        _regcache = {}

        def nreg(v):
            if v not in _regcache:
                _regcache[v] = nc.gpsimd.to_reg(v)
            return _regcache[v]

        GCH = 8  # gather chunk: 8 cols = 1024 idxs (ring holds 2048 descs)

        def gather_chunked(out3, in_ap, colbase, cols):
            # out3: [128, cols, D] SBUF view; idx cols start at colbase*8
            for c0 in range(0, cols, GCH):
                cw = min(GCH, cols - c0)
                nc.gpsimd.dma_gather(
                    out_ap=out3[:, c0 : c0 + cw, :],
                    in_ap=in_ap,
                    idxs_ap=idx_s[:, (colbase + c0) * 8 : (colbase + c0 + cw) * 8],
                    num_idxs=cw * 128,
                    num_idxs_reg=nreg(cw * 128),
                    elem_size=D, queue_num=0)

        def bcast_mid(ap3, k):
            a = ap3.ap
            return bass.AP(tensor=ap3.tensor, offset=ap3.offset,
                           ap=[a[0], a[1], [0, k], a[2]])

        def bcast_last(ap2, k):
            a = ap2.ap
            return bass.AP(tensor=ap2.tensor, offset=ap2.offset,
                           ap=[a[0], a[1], [0, k]])

        def ktree(V4, k):
            # in-place sum over the k axis of V4=[128, nb, k, D] -> [..., 0:1, :]
            while k > 1:
                h = k // 2
                nc.vector.tensor_add(out=V4[:, :, 0:h, :], in0=V4[:, :, 0:h, :],
                                     in1=V4[:, :, h : 2 * h, :])
                if k - 2 * h:
                    nc.vector.tensor_add(
                        out=V4[:, :, 0:1, :], in0=V4[:, :, 0:1, :],
                        in1=V4[:, :, 2 * h : 2 * h + 1, :])
                k = h
            return V4[:, :, 0:1, :]

        for layer in range(3):
            if layer == 0:
                xtc = nrm.tile([128, NBLK, D], bf, tag="ycomp", name="xtc")
                nc.sync.dma_start(
                    out=xtc[:],
                    in_=x_tab_d[:].rearrange("(b m) d -> m b d", m=128))
                tsv = tab_shard[:].rearrange("(b m) d -> m b d", m=128)
                nc.sync.dma_start(out=tsv, in_=xtc[:])
                xsb = ys.tile([128, NBLK, D], bf, tag="y", name="xsb0")
                gather_chunked(xsb[:], x_tab_d[:], OFF0 // 8, NBLK)

            # ---- normalize own shard (dst side) ----
            junk = nrm.tile([128, NBLK, D], bf, tag="ycomp", name="junk")
            nc.vector.tensor_mul(out=junk[:], in0=xsb[:], in1=xsb[:])
            h = D // 2
            while h >= 16:
                nc.vector.tensor_add(out=junk[:, :, 0:h], in0=junk[:, :, 0:h],
                                     in1=junk[:, :, h : 2 * h])
                h //= 2
            nf = nrm.tile([128, NBLK], f32, tag="nf", name="nf")
            nc.vector.tensor_reduce(out=nf[:], in_=junk[:, :, 0:16],
                                    axis=mybir.AxisListType.X, op=Alu.add)
            rstd = nrm.tile([128, NBLK], f32, tag="rstd")
            nc.scalar.activation(
                out=rstd[:], in_=nf[:],
                func=Act.Sqrt, bias=eps_s[:], scale=1.0)
            nc.vector.reciprocal(out=rstd[:], in_=rstd[:])
            xn = nrm.tile([128, NBLK, D], bf, tag="xn")
            nc.vector.tensor_tensor(out=xn[:], in0=xsb[:],
                                    in1=bcast_last(rstd[:], D), op=Alu.mult)

            # ---- AllGather compact raw table ----
            tab_full = tab_fulls[layer]
            nc.gpsimd.collective_compute(
                "AllGather", Alu.bypass, replica_groups=[list(range(NCORE))],
                ins=[tab_shard[:]], outs=[tab_full[:]],
            )

            y_new = ys.tile([128, NBLK, D], bf, tag="y", name="ynew")
            ZallA = sm.tile([128, NBLK], f32, tag="ZallA", name="ZallA")
            ZallB = sm.tile([128, NBLK], f32, tag="ZallB", name="ZallB")

            def phase1(call, kx, colsX, col0, base, gtag):
                # gather + dot/sumsq products + shared halving tree
                t = gat.tile([128, CHMAX, D], bf, tag="t" + gtag,
                             name="t")[:, 0:colsX, :]
                gather_chunked(t, tab_full[base : base + HALF, :],
                               col0, colsX)
                T = tq.tile([128, CHMAX, 2, D], bf, tag="T",
                            name="T")[:, 0:colsX, :, :]
                xnsl = xn[:, call["b0"] : call["b0"] + call["nb"], :]
                nc.vector.tensor_tensor(
                    out=T[:, :, 0, :].rearrange("p (b k) d -> p b k d", k=kx),
                    in0=t.rearrange("p (b k) d -> p b k d", k=kx),
                    in1=bcast_mid(xnsl, kx), op=Alu.mult)
                nc.vector.tensor_mul(out=T[:, :, 1, :], in0=t, in1=t)
                h = D // 2
                while h >= 16:
                    nc.vector.tensor_add(out=T[:, :, :, 0:h],
                                         in0=T[:, :, :, 0:h],
                                         in1=T[:, :, :, h : 2 * h])
                    h //= 2
                qf = tq.tile([128, CHMAX, 2], f32, tag="qf",
                             name="qf", bufs=4)[:, 0:colsX, :]
                nc.vector.tensor_reduce(out=qf, in_=T[:, :, :, 0:16],
                                        axis=mybir.AxisListType.X,
                                        op=Alu.add)
                return t, qf

            def sqrtQ(qf, colsX, gtag):
                Q = qf[:, :, 1:2].rearrange("p c one -> p (c one)")
                R = sm.tile([128, CHMAX], f32, tag="R" + gtag,
                            name="R")[:, 0:colsX]
                nc.scalar.activation(out=R, in_=Q, func=Act.Sqrt,
                                     bias=eps_s[:], scale=1.0)
                return R

            def alphaE(qf, R, colsX, col0, gtag):
                P = qf[:, :, 0:1].rearrange("p c one -> p (c one)")
                nc.vector.reciprocal(out=R, in_=R)
                AL = sm.tile([128, CHMAX], f32, tag="A" + gtag,
                             name="AL")[:, 0:colsX]
                nc.vector.tensor_mul(out=AL, in0=P, in1=R)
                nc.vector.scalar_tensor_tensor(
                    out=AL, in0=AL, scalar=beta_s[:, layer : layer + 1],
                    in1=mask_s[:, col0 : col0 + colsX],
                    op0=Alu.mult, op1=Alu.add)
                return AL

            def expE(AL, colsX, gtag):
                # E duplicated x2 so downstream broadcasts keep a packed
                # last dim (2x DVE mode)
                E2 = sm.tile([128, CHMAX, 2], bf, tag="E" + gtag,
                             name="E2")[:, 0:colsX, :]
                nc.scalar.activation(
                    out=E2, in_=bass.AP(tensor=AL.tensor, offset=AL.offset,
                                        ap=[AL.ap[0], AL.ap[1], [0, 2]]),
                    func=Act.Exp)
                return E2

            def vphase(call, t, E2, kx, colsX, Zall):
                nc.vector.tensor_reduce(
                    out=Zall[:, call["b0"] : call["b0"] + call["nb"]],
                    in_=E2[:, :, 0].rearrange("p (b k) -> p b k", k=kx),
                    axis=mybir.AxisListType.X, op=Alu.add)
                V = vp.tile([128, CHMAX, D], bf, tag="V",
                            name="V")[:, 0:colsX, :]
                e_ap = bass.AP(tensor=E2.tensor, offset=E2.offset,
                               ap=[E2.ap[0], E2.ap[1], [0, D // 2],
                                   E2.ap[2]])
                nc.vector.tensor_tensor(
                    out=V.rearrange("p c (e two) -> p c e two", two=2),
                    in0=t.rearrange("p c (e two) -> p c e two", two=2),
                    in1=e_ap, op=Alu.mult)
                Vs = ktree(V.rearrange("p (b k) d -> p b k d", k=kx), kx)
                return Vs

            # process calls in pairs: Sqrt/Exp activations batch across the
            # pair, halving activation-table reloads
            allcalls = meta["calls"]
            for i0 in range(0, len(allcalls), 1):
                pair = allcalls[i0 : i0 + 1]
                st = []
                for call in pair:
                    cA = (call["colsA"], call["colA0"], 0, "gA")
                    cB = (call["colsB"], CA + call["colB0"], HALF, "gB")
                    tA, qfA = phase1(call, call["ka"], *cA)
                    tB, qfB = phase1(call, call["kb"], *cB)
                    st.append([call, cA, cB, tA, qfA, tB, qfB, None] * 1)
                for s in st:
                    call, cA, cB, tA, qfA, tB, qfB, _ = s
                    s[7] = (sqrtQ(qfA, cA[0], "gA"), sqrtQ(qfB, cB[0], "gB"))
                es = []
                for s in st:
                    call, cA, cB, tA, qfA, tB, qfB, (RA, RB) = s
                    ALA = alphaE(qfA, RA, cA[0], cA[1], "gA")
                    ALB = alphaE(qfB, RB, cB[0], cB[1], "gB")
                    es.append((ALA, ALB))
                for s, (ALA, ALB) in zip(st, es):
                    call, cA, cB = s[0], s[1], s[2]
                    s[7] = (expE(ALA, cA[0], "gA"), expE(ALB, cB[0], "gB"))
                for s in st:
                    call, cA, cB, tA, qfA, tB, qfB, (EA2, EB2) = s
                    b0, nb = call["b0"], call["nb"]
                    VA = vphase(call, tA, EA2, call["ka"], cA[0], ZallA)
                    VB = vphase(call, tB, EB2, call["kb"], cB[0], ZallB)
                    ysl = y_new[:, b0 : b0 + nb, :]
                    nc.vector.tensor_add(
                        out=ysl, in0=VA.rearrange("p b one d -> p (b one) d"),
                        in1=VB.rearrange("p b one d -> p (b one) d"))
                    nc.vector.scalar_tensor_tensor(
                        out=ysl, in0=xsb[:, b0 : b0 + nb, :],
                        scalar=expb_s[:, layer : layer + 1], in1=ysl,
                        op0=Alu.mult, op1=Alu.add)

            # ---- batched denominator ----
            Zt = sm.tile([128, NBLK], f32, tag="Zt", name="Zt")
            ebl = expb_s[:, layer : layer + 1]
            ebl_b = bass.AP(tensor=ebl.tensor, offset=ebl.offset,
                            ap=[ebl.ap[0], [0, NBLK]])
            nc.vector.tensor_add(out=Zt[:], in0=ZallA[:], in1=ZallB[:])
            nc.vector.tensor_tensor(out=Zt[:], in0=Zt[:], in1=ebl_b, op=Alu.add)
            nc.vector.reciprocal(out=Zt[:], in_=Zt[:])
            nc.vector.tensor_tensor(out=y_new[:], in0=y_new[:],
                                    in1=bcast_last(Zt[:], D), op=Alu.mult)

            # ---- repack perm -> compact; feed next AG / output ----
            ydv = y_dram[:].rearrange("(b m) d -> m b d", m=128)
            nc.sync.dma_start(out=ydv, in_=y_new[:])
            ycomp = nrm.tile([128, NBLK, D], bf, tag="ycomp", name="ycomp")
            gather_chunked(ycomp[:], y_dram[:], OFFR // 8, NBLK)
            if layer < 2:
                tsv = tab_shard[:].rearrange("(b m) d -> m b d", m=128)
                nc.sync.dma_start(out=tsv, in_=ycomp[:])
                xsb = y_new
            else:
                yov = y_d[:].rearrange("(b m) d -> m b d", m=128)
                nc.sync.dma_start(out=yov, in_=ycomp[:])
    nc.compile()
    return nc


_CACHE = {}


def prepare(x, edge_index, beta1, beta2, beta3):
    import ml_dtypes
    edge_index = np.asarray(edge_index)
    key = hash(edge_index.tobytes())
    if key not in _CACHE:
        per_core, meta = _plan(edge_index)
        nc = _build_nc(meta)
        _CACHE[key] = (per_core, meta, nc)
    per_core, meta, nc = _CACHE[key]

    x = np.asarray(x, dtype=np.float32)
    betas = np.array([[beta1, beta2, beta3, 0.0]], dtype=np.float32)
    in_maps = []
    for c, pc in enumerate(per_core):
        xt = np.empty((NSH, D), dtype=np.float32)
        xt[:NSH_REAL] = x[c * NSH_REAL : (c + 1) * NSH_REAL]
        xt[NSH_REAL:] = x[c * NSH_REAL]
        in_maps.append(dict(x_tab=xt.astype(ml_dtypes.bfloat16),
                            idxM=pc["idxM"], maskM=pc["maskM"], betas=betas))

    def unshard(ys_list):
        out = np.empty((N, D), dtype=np.float32)
        for c, yv in enumerate(ys_list):
            out[c * NSH_REAL : (c + 1) * NSH_REAL] = np.asarray(
                yv[:NSH_REAL]).astype(np.float32)
        return out

    return nc, in_maps, unshard


def kernel(x, edge_index, beta1, beta2, beta3, trace=False, _ret_info=None):
    nc, in_maps, unshard = prepare(x, edge_index, beta1, beta2, beta3)
    from concourse.bass_utils import run_bass_kernel_spmd

    try:
        r = run_bass_kernel_spmd(nc, in_maps, core_ids=list(range(NCORE)),
                                 trace=trace)
    except ModuleNotFoundError:
        r = run_bass_kernel_spmd(nc, in_maps, core_ids=list(range(NCORE)),
                                 trace=False)
    y = unshard([res["y"] for res in r.results])
    if _ret_info is not None:
        _ret_info["exec_time_ns"] = r.exec_time_ns
        _ret_info["results"] = r
    return y


# revision 54
# speedup vs baseline: 1.2729x; 1.2729x over previous
"""AGNN (3-layer) Trainium2 Bass kernel, 8-core SPMD.

Design:
  dst-node sharding (6250 real nodes/core, NSH=6272 padded positions).
  Gather table = raw node features (bf16, 256B rows) in compact global
  order, AllGathered per layer (12.85 MB). Per-edge dot AND source
  sumsq recomputed on DVE from gathered raw rows (interleaved P/Q
  shared halving tree), so no [xn|xu] 512B rows are needed.
  Self-loops handled analytically (score = beta exactly -> exp(beta)
  terms in numerator/denominator; no slots, no gather).
  Softmax without max-subtraction (|alpha| <= beta).
  Node layout: per-core snake sort by (kA, kB) degree classes over
  {1,2,3,4,6,8,12,16,24,32}; 128-node blocks take per-block max class,
  unified across cores (one NEFF). A/B source halves (4 cores each)
  keep gather indices within int16.
  y stays in SBUF between layers; repack perm->compact via dma_gather
  through DRAM at layer end feeds the next AllGather.
"""

import numpy as np
import sys, os
from contextlib import ExitStack

for _p in ("/opt/trn_rl_repo", "/root/.axon_site/_ro/trn_rl_repo"):
    if os.path.isdir(_p) and _p not in sys.path:
        try:
            import concourse  # noqa
            break
        except Exception:
            sys.path.insert(0, _p)

NCORE = 8
N = 50000
D = 128
NSH_REAL = 6250
NBLK = 49
NSH = NBLK * 128  # 6272
HALF = 4 * NSH  # 25088 rows per source half (cores 0-3 / 4-7)
NTAB = 8 * NSH  # 50176
KC = np.array([1, 2, 3, 4, 6, 8, 12, 16, 24, 32], dtype=np.int64)
CHMAX = 64  # max combined (A+B) gather cols (128 slots each) per call
NEG = np.float32(-1e30)


def _classes(d):
    return KC[np.searchsorted(KC, np.maximum(d, 1))]


def _plan(edge_index):
    src = np.ascontiguousarray(edge_index[0]).astype(np.int64)
    dst = np.ascontiguousarray(edge_index[1]).astype(np.int64)
    core = dst // NSH_REAL
    loc = dst - core * NSH_REAL
    isA = src < 4 * NSH_REAL  # src < 25000

    key = core * NSH_REAL + loc
    dA = np.bincount(key[isA], minlength=N).reshape(NCORE, NSH_REAL)
    dB = np.bincount(key[~isA], minlength=N).reshape(NCORE, NSH_REAL)
    kA = _classes(dA)
    kB = _classes(dB)

    # snake order per core: kA asc, kB asc/desc alternating by kA class idx
    kci = np.zeros(int(KC[-1]) + 1, dtype=np.int64)
    kci[KC] = np.arange(len(KC))
    orders = np.empty((NCORE, NSH_REAL), dtype=np.int64)
    blocksA = np.zeros((NCORE, NBLK), dtype=np.int64)
    blocksB = np.zeros((NCORE, NBLK), dtype=np.int64)
    for c in range(NCORE):
        snake_kb = np.where((kci[kA[c]] % 2) == 1, -kB[c], kB[c])
        o = np.lexsort((snake_kb, kA[c]))
        orders[c] = o
        kAs = np.zeros(NSH, dtype=np.int64)
        kBs = np.zeros(NSH, dtype=np.int64)
        kAs[:NSH_REAL] = kA[c][o]
        kBs[:NSH_REAL] = kB[c][o]
        blocksA[c] = kAs.reshape(NBLK, 128).max(1)
        blocksB[c] = kBs.reshape(NBLK, 128).max(1)
    bA = blocksA.max(0)
    bB = blocksB.max(0)

    colbaseA = np.concatenate([[0], np.cumsum(bA)])
    colbaseB = np.concatenate([[0], np.cumsum(bB)])
    CA, CB = int(colbaseA[-1]), int(colbaseB[-1])
    calls = []
    b = 0
    co = 0
    while b < NBLK:
        e = b + 1
        while e < NBLK and bA[e] == bA[b] and bB[e] == bB[b]:
            e += 1
        npc = max(int(CHMAX // (bA[b] + bB[b])), 1)
        for s in range(b, e, npc):
            nb = min(npc, e - s)
            calls.append(dict(
                ka=int(bA[b]), kb=int(bB[b]), b0=int(s), nb=int(nb),
                colA0=int(colbaseA[s]), colB0=int(colbaseB[s]),
                colsA=int(bA[b] * nb), colsB=int(bB[b] * nb),
                co=int(co),
            ))
            co += int((bA[b] + bB[b]) * nb)
        b = e

    SA, SB = CA * 128, CB * 128
    per_core = []
    for c in range(NCORE):
        o = orders[c]
        pos = np.empty(NSH_REAL, dtype=np.int64)
        pos[o] = np.arange(NSH_REAL)
        m = core == c
        e_src = src[m]
        e_loc = loc[m]
        e_isA = isA[m]
        q = e_loc * 2 + (~e_isA)
        s_idx = np.argsort(q, kind="stable")
        qs = q[s_idx]
        newgrp = np.ones(len(qs), dtype=bool)
        if len(qs) > 1:
            newgrp[1:] = qs[1:] != qs[:-1]
        starts = np.nonzero(newgrp)[0]
        grp_id = np.cumsum(newgrp) - 1
        rank_sorted = np.arange(len(qs)) - starts[grp_id]
        rank = np.empty(len(qs), dtype=np.int64)
        rank[s_idx] = rank_sorted
        p = pos[e_loc]
        blk = p // 128
        part = p - blk * 128
        e_score = e_src // NSH_REAL
        e_srcu = e_src - e_score * NSH_REAL
        # A half = cores 0-3, B half = cores 4-7; B rebased to its region
        rowA = e_score * NSH + e_srcu
        rowB = (e_score - 4) * NSH + e_srcu
        idxA = np.zeros(SA, dtype=np.int64)
        idxB = np.zeros(SB, dtype=np.int64)
        maskA = np.full(SA, NEG, dtype=np.float32)
        maskB = np.full(SB, NEG, dtype=np.float32)
        a = e_isA
        iA = (colbaseA[blk[a]] + rank[a]) * 128 + part[a]
        idxA[iA] = rowA[a]
        maskA[iA] = 0.0
        nb_ = ~a
        iB = (colbaseB[blk[nb_]] + rank[nb_]) * 128 + part[nb_]
        idxB[iB] = rowB[nb_]
        maskB[iB] = 0.0
        idx0 = np.zeros(NSH, dtype=np.int64)
        idx0[:NSH_REAL] = o
        idxR = np.zeros(NSH, dtype=np.int64)
        idxR[:NSH_REAL] = pos

        def wrap16(v):
            return v.reshape(-1, 16).T.astype(np.int16)

        # per-call [A|B] contiguous column layout
        idx_parts, mask_parts = [], []
        for cl in calls:
            a0, a1 = cl["colA0"] * 128, (cl["colA0"] + cl["colsA"]) * 128
            b0_, b1_ = cl["colB0"] * 128, (cl["colB0"] + cl["colsB"]) * 128
            idx_parts += [idxA[a0:a1], idxB[b0_:b1_]]
            mask_parts += [maskA[a0:a1], maskB[b0_:b1_]]
        idx_cat = np.concatenate(idx_parts)
        mask_cat = np.concatenate(mask_parts)
        idxM = np.concatenate(
            [wrap16(idx_cat), wrap16(idx0), wrap16(idxR)], axis=1)
        maskM = mask_cat.reshape(-1, 128).T.astype(np.float32)
        per_core.append(dict(idxM=np.ascontiguousarray(idxM),
                             maskM=np.ascontiguousarray(maskM)))
    meta = dict(CA=CA, CB=CB, W=per_core[0]["idxM"].shape[1], calls=calls)
    return per_core, meta


def _build_nc(meta):
    import concourse.bass as bass
    import concourse.bacc as bacc
    import concourse.tile as tile
    from concourse import mybir, library_config

    f32, bf, i16 = mybir.dt.float32, mybir.dt.bfloat16, mybir.dt.int16
    Alu = mybir.AluOpType
    Act = mybir.ActivationFunctionType

    CA, CB, W = meta["CA"], meta["CB"], meta["W"]
    CM = CA + CB
    OFF0 = CM * 8
    OFFR = OFF0 + NSH // 16

    nc = bacc.Bacc("TRN2", target_bir_lowering=False, debug=False,
                   num_devices=NCORE, num_swdge_queues=1,
                   dynamic_dma_scratch_size=24576)
    x_tab_d = nc.dram_tensor("x_tab", [NSH, D], bf, kind="ExternalInput")
    idxM_d = nc.dram_tensor("idxM", [16, W], i16, kind="ExternalInput")
    maskM_d = nc.dram_tensor("maskM", [128, CM], f32, kind="ExternalInput")
    beta_d = nc.dram_tensor("betas", [1, 4], f32, kind="ExternalInput")
    y_d = nc.dram_tensor("y", [NSH, D], bf, kind="ExternalOutput")

    with ExitStack() as ctx:
        tc = ctx.enter_context(tile.TileContext(nc))
        res = ctx.enter_context(tc.tile_pool(name="res", bufs=1))
        dram = ctx.enter_context(tc.tile_pool(name="dram", bufs=1, space="DRAM"))
        gat = ctx.enter_context(tc.tile_pool(name="gat", bufs=2))
        tq = ctx.enter_context(tc.tile_pool(name="tq", bufs=2))
        sm = ctx.enter_context(tc.tile_pool(name="sm", bufs=4))
        vp = ctx.enter_context(tc.tile_pool(name="vp", bufs=2))
        nrm = ctx.enter_context(tc.tile_pool(name="nrm", bufs=1))
        ys = ctx.enter_context(tc.tile_pool(name="ys", bufs=2))

        idx_s = res.tile([128, W], i16)
        mask_s = res.tile([128, CM], f32)
        beta_s = res.tile([128, 4], f32)
        expb_s = res.tile([128, 4], f32)
        eps_s = res.tile([128, 1], f32)
        tab_shard = dram.tile([NSH, D], bf)
        tab_fulls = [dram.tile([NTAB, D], bf, addr_space="Shared",
                               name=f"tab_full{i}") for i in range(3)]
        y_dram = dram.tile([NSH, D], bf)

        nc.gpsimd.load_library(library_config.mlp)
        for b in range(8):
            nc.sync.dma_start(out=idx_s[16 * b : 16 * (b + 1), :], in_=idxM_d[:])
        nc.sync.dma_start(out=mask_s[:], in_=maskM_d[:])
        bap = beta_d[:]
        nc.gpsimd.dma_start(
            out=beta_s[:],
            in_=bass.AP(tensor=bap.tensor, offset=bap.offset, ap=[[0, 128], [1, 4]]),
        )
        nc.vector.memset(eps_s[:], 1e-20)
        nc.scalar.activation(out=expb_s[:], in_=beta_s[:], func=Act.Exp)

        _regcache = {}

        def nreg(v):
            if v not in _regcache:
                _regcache[v] = nc.gpsimd.to_reg(v)
            return _regcache[v]

        GCH = 8  # gather chunk: 8 cols = 1024 idxs (ring holds 2048 descs)

        def gather_chunked(out3, in_ap, colbase, cols):
            # out3: [128, cols, D] SBUF view; idx cols start at colbase*8
            for c0 in range(0, cols, GCH):
                cw = min(GCH, cols - c0)
                nc.gpsimd.dma_gather(
                    out_ap=out3[:, c0 : c0 + cw, :],
                    in_ap=in_ap,
                    idxs_ap=idx_s[:, (colbase + c0) * 8 : (colbase + c0 + cw) * 8],
                    num_idxs=cw * 128,
                    num_idxs_reg=nreg(cw * 128),
                    elem_size=D, queue_num=0)

        def bcast_mid(ap3, k):
            a = ap3.ap
            return bass.AP(tensor=ap3.tensor, offset=ap3.offset,
                           ap=[a[0], a[1], [0, k], a[2]])

        def bcast_last(ap2, k):
            a = ap2.ap
            return bass.AP(tensor=ap2.tensor, offset=ap2.offset,
                           ap=[a[0], a[1], [0, k]])

        def ktree(V4, k):
            # in-place sum over the k axis of V4=[128, nb, k, D] -> [..., 0:1, :]
            while k > 1:
                h = k // 2
                nc.vector.tensor_add(out=V4[:, :, 0:h, :], in0=V4[:, :, 0:h, :],
                                     in1=V4[:, :, h : 2 * h, :])
                if k - 2 * h:
                    nc.vector.tensor_add(
                        out=V4[:, :, 0:1, :], in0=V4[:, :, 0:1, :],
                        in1=V4[:, :, 2 * h : 2 * h + 1, :])
                k = h
            return V4[:, :, 0:1, :]

        for layer in range(3):
            if layer == 0:
                xtc = nrm.tile([128, NBLK, D], bf, tag="ycomp", name="xtc")
                nc.sync.dma_start(
                    out=xtc[:],
                    in_=x_tab_d[:].rearrange("(b m) d -> m b d", m=128))
                tsv = tab_shard[:].rearrange("(b m) d -> m b d", m=128)
                nc.sync.dma_start(out=tsv, in_=xtc[:])
                xsb = ys.tile([128, NBLK, D], bf, tag="y", name="xsb0")
                gather_chunked(xsb[:], x_tab_d[:], OFF0 // 8, NBLK)

            # ---- normalize own shard (dst side) ----
            junk = nrm.tile([128, NBLK, D], bf, tag="ycomp", name="junk")
            nc.vector.tensor_mul(out=junk[:], in0=xsb[:], in1=xsb[:])
            h = D // 2
            while h >= 16:
                nc.vector.tensor_add(out=junk[:, :, 0:h], in0=junk[:, :, 0:h],
                                     in1=junk[:, :, h : 2 * h])
                h //= 2
            nf = nrm.tile([128, NBLK], f32, tag="nf", name="nf")
            nc.vector.tensor_reduce(out=nf[:], in_=junk[:, :, 0:16],
                                    axis=mybir.AxisListType.X, op=Alu.add)
            rstd = nrm.tile([128, NBLK], f32, tag="rstd")
            nc.scalar.activation(
                out=rstd[:], in_=nf[:],
                func=Act.Sqrt, bias=eps_s[:], scale=1.0)
            nc.vector.reciprocal(out=rstd[:], in_=rstd[:])
            xn = nrm.tile([128, NBLK, D], bf, tag="xn")
            nc.vector.tensor_tensor(out=xn[:], in0=xsb[:],
                                    in1=bcast_last(rstd[:], D), op=Alu.mult)

            # ---- AllGather compact raw table ----
            tab_full = tab_fulls[layer]
            nc.gpsimd.collective_compute(
                "AllGather", Alu.bypass, replica_groups=[list(range(NCORE))],
                ins=[tab_shard[:]], outs=[tab_full[:]],
            )

            y_new = ys.tile([128, NBLK, D], bf, tag="y", name="ynew")
            ZallA = sm.tile([128, NBLK], f32, tag="ZallA", name="ZallA")
            ZallB = sm.tile([128, NBLK], f32, tag="ZallB", name="ZallB")

            for call in meta["calls"]:
                ka, kb, b0, nb = call["ka"], call["kb"], call["b0"], call["nb"]
                colsA, colsB, co = call["colsA"], call["colsB"], call["co"]
                cols = colsA + colsB

                # one combined [A|B] tile per call: shared tree/softmax/V ops
                t = gat.tile([128, CHMAX, D], bf, tag="t",
                             name="t")[:, 0:cols, :]
                gather_chunked(t[:, 0:colsA, :], tab_full[0:HALF, :],
                               co, colsA)
                gather_chunked(t[:, colsA:cols, :], tab_full[HALF:NTAB, :],
                               co + colsA, colsB)
                T = tq.tile([128, CHMAX, 2, D], bf, tag="T", name="T",
                            bufs=1)[:, 0:cols, :, :]
                xnsl = xn[:, b0 : b0 + nb, :]
                nc.vector.tensor_tensor(
                    out=T[:, 0:colsA, 0, :].rearrange(
                        "p (b k) d -> p b k d", k=ka),
                    in0=t[:, 0:colsA, :].rearrange(
                        "p (b k) d -> p b k d", k=ka),
                    in1=bcast_mid(xnsl, ka), op=Alu.mult)
                nc.vector.tensor_tensor(
                    out=T[:, colsA:cols, 0, :].rearrange(
                        "p (b k) d -> p b k d", k=kb),
                    in0=t[:, colsA:cols, :].rearrange(
                        "p (b k) d -> p b k d", k=kb),
                    in1=bcast_mid(xnsl, kb), op=Alu.mult)
                nc.vector.tensor_mul(out=T[:, :, 1, :], in0=t, in1=t)
                h = D // 2
                while h >= 16:
                    nc.vector.tensor_add(out=T[:, :, :, 0:h],
                                         in0=T[:, :, :, 0:h],
                                         in1=T[:, :, :, h : 2 * h])
                    h //= 2
                qf = tq.tile([128, CHMAX, 2], f32, tag="qf",
                             name="qf", bufs=2)[:, 0:cols, :]
                nc.vector.tensor_reduce(out=qf, in_=T[:, :, :, 0:16],
                                        axis=mybir.AxisListType.X,
                                        op=Alu.add)
                # alpha = beta * P * rsqrt(Q) + mask; E duplicated x2 so the
                # V-mult broadcast keeps a packed last dim (2x DVE mode)
                Q = qf[:, :, 1:2].rearrange("p c one -> p (c one)")
                P = qf[:, :, 0:1].rearrange("p c one -> p (c one)")
                R = sm.tile([128, CHMAX], f32, tag="R", name="R")[:, 0:cols]
                nc.scalar.activation(out=R, in_=Q, func=Act.Sqrt,
                                     bias=eps_s[:], scale=1.0)
                nc.vector.reciprocal(out=R, in_=R)
                AL = sm.tile([128, CHMAX], f32, tag="A", name="AL")[:, 0:cols]
                nc.vector.tensor_mul(out=AL, in0=P, in1=R)
                nc.vector.scalar_tensor_tensor(
                    out=AL, in0=AL, scalar=beta_s[:, layer : layer + 1],
                    in1=mask_s[:, co : co + cols],
                    op0=Alu.mult, op1=Alu.add)
                E2 = sm.tile([128, CHMAX, 2], bf, tag="E",
                             name="E2")[:, 0:cols, :]
                nc.scalar.activation(
                    out=E2, in_=bass.AP(tensor=AL.tensor, offset=AL.offset,
                                        ap=[AL.ap[0], AL.ap[1], [0, 2]]),
                    func=Act.Exp)
                nc.vector.tensor_reduce(
                    out=ZallA[:, b0 : b0 + nb],
                    in_=E2[:, 0:colsA, 0].rearrange("p (b k) -> p b k", k=ka),
                    axis=mybir.AxisListType.X, op=Alu.add)
                nc.vector.tensor_reduce(
                    out=ZallB[:, b0 : b0 + nb],
                    in_=E2[:, colsA:cols, 0].rearrange(
                        "p (b k) -> p b k", k=kb),
                    axis=mybir.AxisListType.X, op=Alu.add)
                V = vp.tile([128, CHMAX, D], bf, tag="V",
                            name="V")[:, 0:cols, :]
                e_ap = bass.AP(tensor=E2.tensor, offset=E2.offset,
                               ap=[E2.ap[0], E2.ap[1], [0, D // 2],
                                   E2.ap[2]])
                nc.vector.tensor_tensor(
                    out=V.rearrange("p c (e two) -> p c e two", two=2),
                    in0=t.rearrange("p c (e two) -> p c e two", two=2),
                    in1=e_ap, op=Alu.mult)
                VA = ktree(V[:, 0:colsA, :].rearrange(
                    "p (b k) d -> p b k d", k=ka), ka)
                VB = ktree(V[:, colsA:cols, :].rearrange(
                    "p (b k) d -> p b k d", k=kb), kb)
                ysl = y_new[:, b0 : b0 + nb, :]
                nc.vector.tensor_add(
                    out=ysl, in0=VA.rearrange("p b one d -> p (b one) d"),
                    in1=VB.rearrange("p b one d -> p (b one) d"))
                nc.vector.scalar_tensor_tensor(
                    out=ysl, in0=xsb[:, b0 : b0 + nb, :],
                    scalar=expb_s[:, layer : layer + 1], in1=ysl,
                    op0=Alu.mult, op1=Alu.add)

            # ---- batched denominator ----
            Zt = sm.tile([128, NBLK], f32, tag="Zt", name="Zt")
            ebl = expb_s[:, layer : layer + 1]
            ebl_b = bass.AP(tensor=ebl.tensor, offset=ebl.offset,
                            ap=[ebl.ap[0], [0, NBLK]])
            nc.vector.tensor_add(out=Zt[:], in0=ZallA[:], in1=ZallB[:])
            nc.vector.tensor_tensor(out=Zt[:], in0=Zt[:], in1=ebl_b, op=Alu.add)
            nc.vector.reciprocal(out=Zt[:], in_=Zt[:])
            nc.vector.tensor_tensor(out=y_new[:], in0=y_new[:],
                                    in1=bcast_last(Zt[:], D), op=Alu.mult)

            # ---- repack perm -> compact; feed next AG / output ----
            ydv = y_dram[:].rearrange("(b m) d -> m b d", m=128)
            nc.sync.dma_start(out=ydv, in_=y_new[:])
            ycomp = nrm.tile([128, NBLK, D], bf, tag="ycomp", name="ycomp")
            gather_chunked(ycomp[:], y_dram[:], OFFR // 8, NBLK)
            if layer < 2:
                tsv = tab_shard[:].rearrange("(b m) d -> m b d", m=128)
                nc.sync.dma_start(out=tsv, in_=ycomp[:])
                xsb = y_new
            else:
                yov = y_d[:].rearrange("(b m) d -> m b d", m=128)
                nc.sync.dma_start(out=yov, in_=ycomp[:])
    nc.compile()
    return nc


_CACHE = {}


def prepare(x, edge_index, beta1, beta2, beta3):
    import ml_dtypes
    edge_index = np.asarray(edge_index)
    key = hash(edge_index.tobytes())
    if key not in _CACHE:
        per_core, meta = _plan(edge_index)
        nc = _build_nc(meta)
        _CACHE[key] = (per_core, meta, nc)
    per_core, meta, nc = _CACHE[key]

    x = np.asarray(x, dtype=np.float32)
    betas = np.array([[beta1, beta2, beta3, 0.0]], dtype=np.float32)
    in_maps = []
    for c, pc in enumerate(per_core):
        xt = np.empty((NSH, D), dtype=np.float32)
        xt[:NSH_REAL] = x[c * NSH_REAL : (c + 1) * NSH_REAL]
        xt[NSH_REAL:] = x[c * NSH_REAL]
        in_maps.append(dict(x_tab=xt.astype(ml_dtypes.bfloat16),
                            idxM=pc["idxM"], maskM=pc["maskM"], betas=betas))

    def unshard(ys_list):
        out = np.empty((N, D), dtype=np.float32)
        for c, yv in enumerate(ys_list):
            out[c * NSH_REAL : (c + 1) * NSH_REAL] = np.asarray(
                yv[:NSH_REAL]).astype(np.float32)
        return out

    return nc, in_maps, unshard


def kernel(x, edge_index, beta1, beta2, beta3, trace=False, _ret_info=None):
    nc, in_maps, unshard = prepare(x, edge_index, beta1, beta2, beta3)
    from concourse.bass_utils import run_bass_kernel_spmd

    try:
        r = run_bass_kernel_spmd(nc, in_maps, core_ids=list(range(NCORE)),
                                 trace=trace)
    except ModuleNotFoundError:
        r = run_bass_kernel_spmd(nc, in_maps, core_ids=list(range(NCORE)),
                                 trace=False)
    y = unshard([res["y"] for res in r.results])
    if _ret_info is not None:
        _ret_info["exec_time_ns"] = r.exec_time_ns
        _ret_info["results"] = r
    return y
